# revision 3
# baseline (speedup 1.0000x reference)
"""Longformer sliding-window self-attention on 8 Trainium2 NeuronCores.

Problem: hidden [1, 8192, 768] -> QKV projections (768x768 each) ->
12-head sliding-window attention (one-sided window 256) -> ctx [1, 8192, 768].

Sharding: sequence-parallel across 8 cores. Each core owns 1024 query
positions and recomputes K/V projections over its 1024+2*256 halo-extended
slice (host passes the transposed, zero-padded hidden slice per core).

Per-core device program (all matmuls fp32r unless noted):
  - qT [768,1024], kT [768,1536] feature-major projections (W.T @ hT).
  - v' [1536, 12*65] sequence-major projection with a ones-column per head
    (fused softmax denominator row); bias+padding handled by a K=1 matmul
    against the position-validity row.
  - Per (256-query tile x head): scores^T [768k, 256q] in PSUM via 6
    K=64 matmuls; single ACT exp -> bf16 probs; multiply by a precomputed
    0/1 band*boundary mask (DVE); 6 accumulating bf16 PV matmuls ->
    ctx' [65, 256] where row 64 is the softmax denominator.
  - PE-transpose ctx' back to sequence-major, reciprocal + broadcast
    multiply for the normalization, DMA out [1024, 768].
"""
import numpy as np
from contextlib import ExitStack

import concourse.bass as bass
import concourse.bacc as bacc
import concourse.mybir as mybir
from concourse.tile import TileContext
from concourse.bass_utils import run_bass_kernel_spmd
from concourse.masks import make_identity

F32 = mybir.dt.float32
F32R = mybir.dt.float32r
BF16 = mybir.dt.bfloat16

NCORES = 8
S, HID, H, D, W = 8192, 768, 12, 64, 256
SL = S // NCORES            # 1024 queries per core
EXT = SL + 2 * W            # 1536 extended positions (with halo)
KB = HID // 128             # 6 feature blocks
NT = SL // 256              # 4 query tiles of 256
NJ = 6                      # key tiles of 128 per query tile
NST = EXT // 128            # 12 sequence tiles for v'
EXPF = mybir.ActivationFunctionType.Exp
MUL = mybir.AluOpType.mult


def _build():
    nc = bacc.Bacc(
        "TRN2",
        target_bir_lowering=False,
        debug=False,
        num_devices=NCORES,
    )
    hT_d = nc.declare_dram_parameter("hT", [HID, EXT], F32R, isOutput=False)
    wq_d = nc.declare_dram_parameter("wq", [HID, HID], F32R, isOutput=False)
    wk_d = nc.declare_dram_parameter("wk", [HID, HID], F32R, isOutput=False)
    wv_d = nc.declare_dram_parameter("wv", [HID, HID], F32R, isOutput=False)
    bias_d = nc.declare_dram_parameter("biasqk", [128, 2 * KB], F32, isOutput=False)
    pvt_d = nc.declare_dram_parameter("pvt", [128, NST], F32, isOutput=False)
    pvrow_d = nc.declare_dram_parameter("pvrow", [1, EXT], F32R, isOutput=False)
    bvrow_d = nc.declare_dram_parameter("bvrow", [1, HID], F32R, isOutput=False)
    out_d = nc.declare_dram_parameter("out", [SL, HID], F32, isOutput=True)

    with ExitStack() as ctx:
        tc = ctx.enter_context(TileContext(nc))
        pH = ctx.enter_context(tc.tile_pool(name="h", bufs=1))
        pW = ctx.enter_context(tc.tile_pool(name="w", bufs=12))
        pQ = ctx.enter_context(tc.tile_pool(name="q", bufs=1))
        pK = ctx.enter_context(tc.tile_pool(name="k", bufs=1))
        pV = ctx.enter_context(tc.tile_pool(name="v", bufs=1))
        pProb = ctx.enter_context(tc.tile_pool(name="prob", bufs=2))
        pMask = ctx.enter_context(tc.tile_pool(name="mask", bufs=1))
        pCtx = ctx.enter_context(tc.tile_pool(name="ctx", bufs=12))
        pOut = ctx.enter_context(tc.tile_pool(name="outp", bufs=2))
        pMisc = ctx.enter_context(tc.tile_pool(name="misc", bufs=1))
        pRec = ctx.enter_context(tc.tile_pool(name="rec", bufs=2))
        pSc = ctx.enter_context(tc.tile_pool(name="scps", bufs=2, space="PSUM"))
        pPs = ctx.enter_context(tc.tile_pool(name="ps", bufs=2, space="PSUM"))

        # ---- constants / small inputs
        bias_sb = pMisc.tile([128, 2 * KB], F32, tag="bias")
        nc.sync.dma_start(bias_sb[:], bias_d[:])
        pvt_sb = pMisc.tile([128, NST], F32, tag="pvt")
        nc.sync.dma_start(pvt_sb[:], pvt_d[:])
        pvrow_sb = pMisc.tile([1, EXT], F32R, tag="pvrow")
        nc.sync.dma_start(pvrow_sb[:], pvrow_d[:])
        bvrow_sb = pMisc.tile([1, HID], F32R, tag="bvrow")
        nc.sync.dma_start(bvrow_sb[:], bvrow_d[:])

        ident = pMisc.tile([65, 65], F32, tag="ident")
        make_identity(nc, ident[:])

        # per-q-tile 0/1 masks in scores^T layout [k-part, (j, c)]:
        # band: valid iff 0 <= p + 128*j - c <= 2*W; then multiply by the
        # sequence-boundary validity of each key position (broadcast over c).
        masks = []
        for t in range(NT):
            mk = pMask.tile([128, NJ * 256], BF16, tag=f"m{t}", name=f"mask{t}")
            nc.gpsimd.memset(mk[:], 1.0)
            nc.gpsimd.affine_select(
                out=mk[:], in_=mk[:], compare_op=mybir.AluOpType.is_ge,
                fill=0.0, base=0, pattern=[[128, NJ], [-1, 256]],
                channel_multiplier=1)
            nc.gpsimd.affine_select(
                out=mk[:], in_=mk[:], compare_op=mybir.AluOpType.is_ge,
                fill=0.0, base=2 * W, pattern=[[-128, NJ], [1, 256]],
                channel_multiplier=-1)
            mv = mk[:].rearrange("p (j c) -> p j c", j=NJ)
            pvv = pvt_sb[:, 2 * t: 2 * t + NJ].rearrange("p (j c) -> p j c", c=1)
            _, pvb = bass.broadcast_tensor_aps(mv, pvv)
            nc.vector.tensor_tensor(mv, mv, pvb, MUL)
            masks.append(mk)

        # ---- weight & hidden DMAs (row-tiles of 128 partitions)
        wq_t = []
        for k in range(KB):
            w = pW.tile([128, HID], F32R, tag="w")
            nc.sync.dma_start(w[:], wq_d[k * 128:(k + 1) * 128, :])
            wq_t.append(w)
        h_t = [[None] * 3 for _ in range(KB)]
        for c in range(3):
            for k in range(KB):
                ht = pH.tile([128, 512], F32R, tag=f"h{k}_{c}")
                nc.sync.dma_start(ht[:], hT_d[k * 128:(k + 1) * 128,
                                              c * 512:(c + 1) * 512])
                h_t[k][c] = ht
        wk_t = []
        for k in range(KB):
            w = pW.tile([128, HID], F32R, tag="w")
            nc.sync.dma_start(w[:], wk_d[k * 128:(k + 1) * 128, :])
            wk_t.append(w)

        qT_t = [pQ.tile([128, SL], F32R, tag=f"q{m}", name=f"qT{m}") for m in range(KB)]
        kT_t = [pK.tile([128, EXT], F32R, tag=f"k{m}", name=f"kT{m}") for m in range(KB)]
        v_t = [pV.tile([128, H * 65], BF16, tag=f"v{st}", name=f"vp{st}") for st in range(NST)]

        def emit_qproj(c4):
            eo = W + c4 * 256
            ch, off = eo // 512, eo % 512
            for m in range(KB):
                ps = pPs.tile([128, 256], F32, tag="ps")
                for k in range(KB):
                    nc.tensor.matmul(
                        ps[:], lhsT=wq_t[k][:, m * 128:(m + 1) * 128],
                        rhs=h_t[k][ch][:, off:off + 256],
                        start=(k == 0), stop=(k == KB - 1))
                nc.vector.tensor_scalar_add(
                    qT_t[m][:, c4 * 256:(c4 + 1) * 256], ps[:],
                    bias_sb[:, m:m + 1])

        def emit_kproj(c):
            for m in range(KB):
                ps = pPs.tile([128, 512], F32, tag="ps")
                for k in range(KB):
                    nc.tensor.matmul(
                        ps[:], lhsT=wk_t[k][:, m * 128:(m + 1) * 128],
                        rhs=h_t[k][c][:, :],
                        start=(k == 0), stop=(k == KB - 1))
                nc.vector.tensor_scalar_add(
                    kT_t[m][:, c * 512:(c + 1) * 512], ps[:],
                    bias_sb[:, KB + m:KB + m + 1])

        def emit_vproj(st):
            vt = v_t[st]
            vv = vt[:].rearrange("p (h x) -> p h x", x=65)
            nc.gpsimd.memset(vv[:, :, 64:65], 1.0)
            ch, off = st // 4, (st % 4) * 128
            for (f0, nf) in ((0, 512), (512, 256)):
                ps = pPs.tile([128, nf], F32, tag="ps")
                for k in range(KB):
                    nc.tensor.matmul(
                        ps[:], lhsT=h_t[k][ch][:, off:off + 128],
                        rhs=wv_t[k][:, f0:f0 + nf],
                        start=(k == 0), stop=False)
                nc.tensor.matmul(
                    ps[:], lhsT=pvrow_sb[0:1, st * 128:(st + 1) * 128],
                    rhs=bvrow_sb[0:1, f0:f0 + nf], start=False, stop=True)
                nc.vector.tensor_copy(
                    vv[:, f0 // 64:(f0 + nf) // 64, 0:64],
                    ps[:].rearrange("p (h x) -> p h x", x=64))

        def emit_scores(t, h):
            kb, po = h // 2, (h % 2) * 64
            sc = pSc.tile([128, NJ * 256], F32, tag="sc")
            for j in range(NJ):
                k0 = t * 256 + j * 128
                nc.tensor.matmul(
                    sc[:, j * 256:(j + 1) * 256],
                    lhsT=kT_t[kb][po:po + 64, k0:k0 + 128],
                    rhs=qT_t[kb][po:po + 64, t * 256:(t + 1) * 256],
                    start=True, stop=True)
            pr = pProb.tile([128, NJ * 256], BF16, tag="pr")
            nc.scalar.activation(pr[:], sc[:], EXPF)
            nc.vector.tensor_mul(pr[:], pr[:], masks[t][:])
            return pr

        def emit_pv(t, h, prm):
            cx = pPs.tile([65, 256], F32, tag="ps")
            for j in range(NJ):
                nc.tensor.matmul(
                    cx[:], lhsT=v_t[2 * t + j][:, h * 65:(h + 1) * 65],
                    rhs=prm[:, j * 256:(j + 1) * 256],
                    start=(j == 0), stop=(j == NJ - 1))
            cs = pCtx.tile([65, 256], F32, tag="ctx")
            nc.vector.tensor_copy(cs[:], cx[:])
            return cs

        def emit_attn(t):
            LOOK = 2
            prs = {}
            css = []
            for i in range(H + LOOK):
                if i < H:
                    prs[i] = emit_scores(t, i)
                if i >= LOOK:
                    css.append(emit_pv(t, i - LOOK, prs.pop(i - LOOK)))
            return css

        def emit_finish(t, css):
            for half in range(2):
                ob = pOut.tile([128, HID], F32, tag="out")
                for g in range(2):
                    pt = pPs.tile([128, 390], F32, tag="ps")
                    for hh in range(6):
                        nc.tensor.transpose(
                            pt[:, hh * 65:(hh + 1) * 65],
                            css[g * 6 + hh][:, half * 128:(half + 1) * 128],
                            ident[:])
                    ptv = pt[:].rearrange("p (h x) -> p h x", h=6)
                    rc = pRec.tile([128, 6], F32, tag="rec")
                    nc.vector.reciprocal(
                        rc[:].rearrange("p (h x) -> p h x", x=1),
                        ptv[:, :, 64:65])
                    ov = ob[:].rearrange("p (h x) -> p h x", h=H)[:, g * 6:(g + 1) * 6, :]
                    i0 = ptv[:, :, 0:64]
                    _, rb = bass.broadcast_tensor_aps(
                        i0, rc[:].rearrange("p (h x) -> p h x", x=1))
                    nc.vector.tensor_tensor(ov, i0, rb, MUL)
                r0 = t * 256 + half * 128
                nc.sync.dma_start(out_d[r0:r0 + 128, :], ob[:])

        # ---- schedule: early slices first so attention overlaps projections
        for c4 in range(NT):
            emit_qproj(c4)
        emit_kproj(0)
        emit_kproj(1)
        wv_t = []
        for k in range(KB):
            w = pW.tile([128, HID], F32R, tag="w")
            nc.sync.dma_start(w[:], wv_d[k * 128:(k + 1) * 128, :])
            wv_t.append(w)
        for st in range(6):
            emit_vproj(st)
        emit_finish(0, emit_attn(0))
        emit_kproj(2)
        emit_vproj(6)
        emit_vproj(7)
        emit_finish(1, emit_attn(1))
        emit_vproj(8)
        emit_vproj(9)
        emit_finish(2, emit_attn(2))
        emit_vproj(10)
        emit_vproj(11)
        emit_finish(3, emit_attn(3))

    nc.compile()
    return nc


_NC = None


def _get_nc():
    global _NC
    if _NC is None:
        _NC = _build()
    return _NC


def _prepare_in_maps(hidden_states, Wq, bq, Wk, bk, Wv, bv):
    hidden_states = np.asarray(hidden_states, dtype=np.float32)
    Wq = np.asarray(Wq, dtype=np.float32)
    Wk = np.asarray(Wk, dtype=np.float32)
    Wv = np.asarray(Wv, dtype=np.float32)
    bq = np.asarray(bq, dtype=np.float32)
    bk = np.asarray(bk, dtype=np.float32)
    bv = np.asarray(bv, dtype=np.float32)

    scale = 1.0 / np.sqrt(D).astype(np.float32)
    hT = np.ascontiguousarray(hidden_states.reshape(S, HID).T)  # [768, 8192]
    wq_s = np.ascontiguousarray(Wq * scale)
    biasqk = np.concatenate(
        [(bq * scale).reshape(KB, 128).T, bk.reshape(KB, 128).T], axis=1)
    biasqk = np.ascontiguousarray(biasqk, dtype=np.float32)
    bvrow = np.ascontiguousarray(bv.reshape(1, HID))

    in_maps = []
    for c in range(NCORES):
        lo, hi = c * SL - W, c * SL + SL + W
        padl, padr = max(0, -lo), max(0, hi - S)
        hT_c = np.zeros((HID, EXT), dtype=np.float32)
        hT_c[:, padl:EXT - padr] = hT[:, lo + padl:hi - padr]
        pv = np.zeros(EXT, dtype=np.float32)
        pv[padl:EXT - padr] = 1.0
        in_maps.append(dict(
            hT=hT_c,
            wq=wq_s, wk=Wk, wv=Wv,
            biasqk=biasqk,
            pvt=np.ascontiguousarray(pv.reshape(NST, 128).T),
            pvrow=np.ascontiguousarray(pv.reshape(1, EXT)),
            bvrow=bvrow,
        ))
    return in_maps


def kernel(hidden_states, Wq, bq, Wk, bk, Wv, bv):
    nc = _get_nc()
    in_maps = _prepare_in_maps(hidden_states, Wq, bq, Wk, bk, Wv, bv)
    res = run_bass_kernel_spmd(nc, in_maps, list(range(NCORES)))
    out = np.concatenate([res.results[c]["out"] for c in range(NCORES)], axis=0)
    return out.reshape(1, S, HID)
